# revision 11
# baseline (speedup 1.0000x reference)
"""Causal multi-head attention on 8 TRN2 NeuronCores.

Problem: x[4,2048,1024], w_attn[1024,3072], w_proj[1024,1024],
16 heads x 64 dim, causal softmax(QK^T/8)V then output projection.

Sharding: 4-way batch x 2-way head-half. Core c handles batch c//2 and
heads (c%2)*8 .. (c%2)*8+8. Each core computes a partial y^T (its head
half's contribution to the output projection); the host sums the two
partials per batch and transposes.

Per-core layout strategy (all matmuls fp32r, 1 cycle/row at N>=256):
 - host feeds x^T [1024, 2048] (c_in-major)
 - QKV projection: Q^T,K^T computed feature-major [512, T]; V computed
   token-major [T, 512] (so no on-device transposes anywhere)
 - attention computed transposed: S^T[k,q] = (K^T).T-slices @ Q^T with
   K=64 contraction run as PE 64x128 row-tile pairs (two heads at
   partition bases 0/64 execute concurrently on T0/T8)
 - P = exp(S^T/8) on ACT straight out of PSUM; causal masking by
   multiplying diagonal 512-chunks with precomputed masks
 - PV: O^T[d,q] accumulated over key tiles with stationary [V_h | 1]
   (M=65) so PSUM row 64 carries the softmax denominators for free
 - normalize via DVE reciprocal + log-doubling partition broadcast
 - projection: y^T partial = w_proj_slice.T-rows @ O^T
"""

import dataclasses
import numpy as np
from contextlib import ExitStack

import concourse.bass as bass
import concourse.tile as tile
from concourse import bacc, mybir
from concourse.bass_utils import run_bass_kernel_spmd

f32 = mybir.dt.float32
f32r = mybir.dt.float32r
EXP = mybir.ActivationFunctionType.Exp

B, T, C = 4, 2048, 1024
N_HEAD, HD = 16, 64
HPC = 8            # heads per core
FS = HPC * HD      # 512: per-core feature slice for each of q/k/v
NPAIR = HPC // 2   # 4 head pairs
SCALE = 1.0 / 8.0  # 1/sqrt(64)
N_CORES = 8


def build_nc(tpc=T):
    """Build the single-core Bass program (SPMD: same program all cores)."""
    nck = C // 128          # 8 c_in tiles
    nkt = tpc // 128        # key tiles
    nqc = tpc // 512        # query chunks (512 wide)
    nmt = C // 128          # 8 output-channel tiles

    nc = bacc.Bacc("TRN2", target_bir_lowering=False)
    xt = nc.dram_tensor("xt", [C, tpc], f32r, kind="ExternalInput")
    wq = nc.dram_tensor("wq", [C, FS], f32r, kind="ExternalInput")
    wk = nc.dram_tensor("wk", [C, FS], f32r, kind="ExternalInput")
    wv = nc.dram_tensor("wv", [C, FS], f32r, kind="ExternalInput")
    wp = nc.dram_tensor("wp", [FS, C], f32r, kind="ExternalInput")
    mk = nc.dram_tensor("mk", [128, 4, 512], f32, kind="ExternalInput")
    yt = nc.dram_tensor("yt", [C, tpc], f32, kind="ExternalOutput")

    with tile.TileContext(nc) as tc, ExitStack() as stk:
        # tensors that live across stages
        persist = stk.enter_context(tc.tile_pool(name="persist", bufs=1))
        qT = [persist.tile([128, tpc], f32r, tag=f"qT{p}", name=f"qT{p}") for p in range(NPAIR)]
        kT = [persist.tile([128, tpc], f32r, tag=f"kT{p}", name=f"kT{p}") for p in range(NPAIR)]
        # vhat[kt]: [128 keys, 8 heads, 64 dims + ones column]
        vhat = [persist.tile([128, HPC, HD + 1], f32r, tag=f"vh{t}", name=f"vh{t}")
                for t in range(nkt)]
        mkt = persist.tile([128, 4, 512], f32, tag="mk")
        nc.sync.dma_start(out=mkt, in_=mk[:, :, :])
        ones_f = persist.tile([128, HPC], f32, tag="ones")
        nc.vector.memset(ones_f[:, :], 1.0)

        # ---------------- Stage A: QKV projection ----------------
        with tc.tile_pool(name="xa", bufs=1) as xa, \
             tc.tile_pool(name="wa", bufs=2) as wa, \
             tc.tile_pool(name="wb", bufs=1) as wb, \
             tc.tile_pool(name="psa", bufs=4, space="PSUM") as psa:
            xts = []
            for i in range(nck):
                x_i = xa.tile([128, tpc], f32r, tag=f"x{i}")
                nc.sync.dma_start(out=x_i, in_=xt[i * 128:(i + 1) * 128, :])
                xts.append(x_i)

            # Q^T / K^T feature-major: out[feat, tok]
            for dst, wsrc in ((qT, wq), (kT, wk)):
                for m in range(NPAIR):
                    wt = wa.tile([128, nck, 128], f32r, tag="wqk")
                    nc.sync.dma_start(
                        out=wt,
                        in_=wsrc.rearrange("(a p) f -> p a f", p=128)[
                            :, :, m * 128:(m + 1) * 128])
                    for n in range(tpc // 512):
                        ps = psa.tile([128, 512], f32, tag="ps")
                        for k in range(nck):
                            nc.tensor.matmul(
                                ps[:, :], wt[:, k, :],
                                xts[k][:, n * 512:(n + 1) * 512],
                                start=(k == 0), stop=(k == nck - 1))
                        nc.vector.tensor_copy(
                            dst[m][:, n * 512:(n + 1) * 512], ps[:, :])

            # V token-major: out[tok, feat]; scatter into vhat with ones col
            wvt = wb.tile([128, nck, FS], f32r, tag="wv")
            nc.sync.dma_start(out=wvt, in_=wv.rearrange("(a p) f -> p a f", p=128))
            for t in range(nkt):
                ps = psa.tile([128, FS], f32, tag="ps")
                for k in range(nck):
                    nc.tensor.matmul(
                        ps[:, :], xts[k][:, t * 128:(t + 1) * 128], wvt[:, k, :],
                        start=(k == 0), stop=(k == nck - 1))
                nc.vector.tensor_copy(
                    vhat[t][:, :, 0:HD],
                    ps[:, :].rearrange("p (h d) -> p h d", h=HPC))
                nc.vector.tensor_copy(vhat[t][:, :, HD], ones_f[:, :])

        # ---------------- Stages B+C ----------------
        otp = stk.enter_context(tc.tile_pool(name="ot", bufs=1))
        oT = [otp.tile([128, tpc], f32r, tag=f"oT{p}", name=f"oT{p}") for p in range(NPAIR)]

        with tc.tile_pool(name="pp", bufs=12) as pp, \
             tc.tile_pool(name="rp", bufs=3) as rp, \
             tc.tile_pool(name="psS", bufs=4, space="PSUM") as psS, \
             tc.tile_pool(name="psO", bufs=4, space="PSUM") as psO:
            for p in range(NPAIR):
                for qc in range(nqc):
                    kts = list(range(min(nkt, 4 * (qc + 1))))
                    qsl = slice(qc * 512, (qc + 1) * 512)
                    ptiles = []
                    for kt in kts:
                        ksl = slice(kt * 128, (kt + 1) * 128)
                        prs = []
                        for par in range(2):   # head parity: partitions 0/64
                            row = slice(64 * par, 64 * par + 64)
                            ps = psS.tile([128, 512], f32, tag="s")
                            nc.tensor.matmul(
                                ps[:, :], kT[p][row, ksl], qT[p][row, qsl],
                                start=True, stop=True)
                            pr = pp.tile([128, 512], f32r, tag="P")
                            nc.scalar.activation(pr[:, :], ps[:, :], EXP,
                                                 scale=SCALE)
                            if kt // 4 == qc:  # diagonal chunk: causal mask
                                nc.vector.tensor_mul(pr[:, :], pr[:, :],
                                                     mkt[:, kt % 4, :])
                            prs.append(pr)
                        ptiles.append(prs)
                    # PV accumulation, both heads interleaved
                    po = [psO.tile([128, 512], f32, tag="o", name="po") for _ in range(2)]
                    for kt in kts:
                        for par in range(2):
                            nc.tensor.matmul(
                                po[par][0:HD + 1, :],
                                vhat[kt][:, 2 * p + par, :],
                                ptiles[kt][par][:, :],
                                start=(kt == 0), stop=(kt == kts[-1]))
                    # normalize: O / den, den in psum row 64
                    for par in range(2):
                        rden = rp.tile([1, 512], f32, tag="rden")
                        nc.vector.reciprocal(rden[:, :], po[par][HD:HD + 1, :])
                        bci = rp.tile([64, 512], f32, tag="bci")
                        nc.vector.memset(bci[:, :], 0.0)
                        nc.vector.tensor_copy(bci[0:1, :], rden[:, :])
                        nc.vector.tensor_copy(bci[32:33, :], rden[:, :])
                        bc = rp.tile([64, 512], f32, tag="bc")
                        nc.vector.stream_shuffle(bc[:, :], bci[:, :], [0] * 32)
                        nc.vector.tensor_mul(
                            oT[p][64 * par:64 * par + 64, qsl],
                            po[par][0:HD, :], bc[:, :])

        # ---------------- Stage C: output projection ----------------
        with tc.tile_pool(name="wc", bufs=2) as wc, \
             tc.tile_pool(name="ev", bufs=4) as ev, \
             tc.tile_pool(name="psC", bufs=4, space="PSUM") as psC:
                for m in range(nmt):
                    wpt = wc.tile([128, NPAIR, 128], f32r, tag="wp")
                    nc.sync.dma_start(
                        out=wpt,
                        in_=wp.rearrange("(a p) f -> p a f", p=128)[
                            :, :, m * 128:(m + 1) * 128])
                    for n in range(tpc // 512):
                        ps = psC.tile([128, 512], f32, tag="ps")
                        for j in range(NPAIR):
                            nc.tensor.matmul(
                                ps[:, :], wpt[:, j, :],
                                oT[j][:, n * 512:(n + 1) * 512],
                                start=(j == 0), stop=(j == NPAIR - 1))
                        sb = ev.tile([128, 512], f32, tag="sb")
                        nc.vector.tensor_copy(sb[:, :], ps[:, :])
                        nc.sync.dma_start(
                            out=yt[m * 128:(m + 1) * 128,
                                   n * 512:(n + 1) * 512],
                            in_=sb)
    nc.compile()
    return nc


def _make_masks():
    k = np.arange(128)[:, None]
    q = np.arange(512)[None, :]
    m = np.empty((128, 4, 512), np.float32)
    for j in range(4):
        m[:, j, :] = (q >= k + 128 * j).astype(np.float32)
    return m


_NC_CACHE = {}


def _get_nc(tpc=T):
    if tpc not in _NC_CACHE:
        _NC_CACHE[tpc] = build_nc(tpc)
    return _NC_CACHE[tpc]


def make_in_maps(x, w_attn, w_proj):
    masks = _make_masks()
    in_maps = []
    for core in range(N_CORES):
        b, hh = core // 2, core % 2
        s = slice(hh * FS, (hh + 1) * FS)
        in_maps.append({
            "xt": np.ascontiguousarray(x[b].T),
            "wq": np.ascontiguousarray(w_attn[:, s]),
            "wk": np.ascontiguousarray(w_attn[:, C:][:, s]),
            "wv": np.ascontiguousarray(w_attn[:, 2 * C:][:, s]),
            "wp": np.ascontiguousarray(w_proj[hh * FS:(hh + 1) * FS, :]),
            "mk": masks,
        })
    return in_maps


def kernel(x, w_attn, w_proj):
    nc = _get_nc(T)
    in_maps = make_in_maps(x, w_attn, w_proj)
    res = run_bass_kernel_spmd(nc, in_maps, list(range(N_CORES)))
    y = np.empty((B, T, C), np.float32)
    for b in range(B):
        yt = res.results[2 * b]["yt"] + res.results[2 * b + 1]["yt"]
        y[b] = yt.T
    return y
